# revision 20
# baseline (speedup 1.0000x reference)
"""Trainium2 Bass kernel for nn_ConvLocalAttention (b=8, dim=512, n=2048,
heads=8, dim_head=64, window=128, causal local attention with look_backward=1,
qk rmsnorm, QK_SCALE=8).

Strategy: data-parallel over batch -- one batch element per NeuronCore (8 cores).
All matmuls in bf16. Per core:
  A. load x (int8 + per-[chan, 128-token-block] f32 scales), weights (bf16);
     dequantize x to bf16 on DVE (int8->bf16 copy, then scale multiply)
  B. v projection token-major: vT[n, h, d] (+ ones column for softmax denom)
  C. q,k projections channel-major + qk-rmsnorm:
       ssq per (head, token) via block-diag-ones matmul of q^2 (ACT Square)
       rn = 1/sqrt(ssq) broadcast to channels via PE repeat-matrix matmul
       qh = q * rn ; kh = k * rn * (8*q_scale*k_scale per channel)
  D. local attention per head:
       scores^T[j, i] = kh_block^T @ qh  (key-major, 4 blocks per PSUM group)
       p = exp(scores) (ACT, batched) * band-mask (DVE, bf16)
       PV token-major: out[i, d|sum] = p_half^T @ [vT | 1], two window halves
       accumulate in PSUM; normalize by 1/sum (col 64) -> att[tok, head, d] bf16
  E. transpose att to channel-major via DMA transpose (64 x 128x128 tiles)
  F. out = w_out @ att; quantize PSUM f32 -> int8 per [chan, 128-token-block]
     (absmax via DVE reduce, HW f32->int8 convert rounds to nearest) and emit
     int8 data + f32 scales. Host dequantizes.

Host<->device traffic is the wall-clock bottleneck (axon-tunneled devices,
~45-55 MB/s half-duplex, ~75 ms per dispatch), so the runner:
  - builds + caches ONE jitted shard_map callable (no per-call retrace/XLA),
  - keeps the (replicated) weights resident on device, memoized by content,
  - donates the previous call's output buffers as the NEFF's output operands
    (the kernel fully overwrites them) so no zero-buffers cross the wire,
  - ships x as int8+scales (8.5 MB) and the result as int8+scales (8.7 MB).
"""
import hashlib
import numpy as np
import ml_dtypes

import jax
import jax.numpy as jnp
from jax.sharding import Mesh, PartitionSpec, NamedSharding

try:
    from jax import shard_map as _shard_map

    def shard_map(f, mesh, in_specs, out_specs, check_rep):
        return _shard_map(f, mesh=mesh, in_specs=in_specs, out_specs=out_specs,
                          check_vma=check_rep)
except ImportError:  # older jax
    from jax.experimental.shard_map import shard_map as _shard_map_old

    def shard_map(f, mesh, in_specs, out_specs, check_rep):
        return _shard_map_old(f, mesh=mesh, in_specs=in_specs,
                              out_specs=out_specs, check_rep=check_rep)

import concourse.bass as bass
import concourse.mybir as mybir
import concourse.tile as tile
from concourse import bacc
from concourse.bass2jax import (
    _bass_exec_p,
    install_neuronx_cc_hook,
    partition_id_tensor,
)

F32 = mybir.dt.float32
BF16 = mybir.dt.bfloat16
I8 = mybir.dt.int8
AF = mybir.ActivationFunctionType
ALU = mybir.AluOpType
AX = mybir.AxisListType

H = 8          # heads
D = 64         # dim head
C = 512        # model dim
N = 2048       # seq len
W = 128        # window
NW = N // W    # 16 windows
NT = 4         # n-tiles of 512 tokens
CS = 4         # channel subtiles of 128
B = 8          # batch = n cores

X8 = True      # ship x as int8 + per-block scales
O8 = True      # ship out as int8 + per-block scales

_CACHE = {}


def build_nc():
    if "nc" in _CACHE:
        return _CACHE["nc"]
    nc = bacc.Bacc("TRN2", target_bir_lowering=False, debug=False, num_devices=8)

    if X8:
        # packed: [:, :N] int8 data, [:, N:N+32] per-block bf16 scales (bitcast)
        xq_d = nc.dram_tensor("xq", [C, N + 2 * NW], I8, kind="ExternalInput").ap()
    else:
        x_d = nc.dram_tensor("x", [C, N], BF16, kind="ExternalInput").ap()
    wqk_d = nc.dram_tensor("wqk", [C, 2 * C], BF16, kind="ExternalInput").ap()
    wv_d = nc.dram_tensor("wv", [C, C], BF16, kind="ExternalInput").ap()
    wo_d = nc.dram_tensor("wo", [C, C], BF16, kind="ExternalInput").ap()
    cs_d = nc.dram_tensor("cs", [C, 1], F32, kind="ExternalInput").ap()
    bd_d = nc.dram_tensor("bd", [C, H], BF16, kind="ExternalInput").ap()
    rep_d = nc.dram_tensor("rep", [H, C], BF16, kind="ExternalInput").ap()
    mk_d = nc.dram_tensor("mk", [W, 2 * W], BF16, kind="ExternalInput").ap()
    if O8:
        # packed: [:, :N] int8 data, [:, N:N+32] per-block bf16 scales (bitcast)
        oq_d = nc.dram_tensor("oq", [C, N + 2 * NW], I8, kind="ExternalOutput").ap()
    else:
        out_d = nc.dram_tensor("out", [C, N], BF16, kind="ExternalOutput").ap()

    with tile.TileContext(nc) as tc:
        with tc.tile_pool(name="persist", bufs=1) as pp:
            # persistent SBUF tensors
            xs = [pp.tile([W, N], BF16, name=f"xs{s}") for s in range(CS)]
            wqks = [pp.tile([W, 2 * C], BF16, name=f"wqk{s}") for s in range(CS)]
            wvs = [pp.tile([W, C], BF16, name=f"wv{s}") for s in range(CS)]
            wos = [pp.tile([W, C], BF16, name=f"wo{s}") for s in range(CS)]
            css = [pp.tile([W, 1], F32, name=f"cs{s}") for s in range(CS)]
            bds = [pp.tile([W, H], BF16, name=f"bd{s}") for s in range(CS)]
            mks = pp.tile([W, 2 * W], BF16, name="mk")
            reps = pp.tile([H, C], BF16, name="reps")
            qh = [pp.tile([W, N], BF16, name=f"qh{s}") for s in range(CS)]
            kh = [pp.tile([W, N], BF16, name=f"kh{s}") for s in range(CS)]
            vt = pp.tile([W, NW, H, D + 1], BF16, name="vt")
            att = pp.tile([W, NW, C], BF16, name="att")
            attc = [pp.tile([W, N], BF16, name=f"attc{s}") for s in range(CS)]
            if X8:
                xq8 = [pp.tile([W, N], I8, name=f"xq8{s}") for s in range(CS)]
                xscs = [pp.tile([W, NW], BF16, name=f"xsc{s}") for s in range(CS)]

            # ---- A: input DMAs ----
            for s in range(CS):
                sl = slice(s * W, (s + 1) * W)
                if X8:
                    nc.sync.dma_start(xq8[s][:], xq_d[sl, 0:N])
                    nc.sync.dma_start(
                        xscs[s][:], xq_d[sl, N:N + 2 * NW].bitcast(BF16))
                else:
                    nc.sync.dma_start(xs[s][:], x_d[sl, :])
                nc.sync.dma_start(wqks[s][:], wqk_d[sl, :])
                nc.sync.dma_start(wvs[s][:], wv_d[sl, :])
                nc.sync.dma_start(wos[s][:], wo_d[sl, :])
                nc.sync.dma_start(css[s][:], cs_d[sl, :])
                nc.sync.dma_start(bds[s][:], bd_d[sl, :])
            nc.sync.dma_start(mks[:], mk_d)
            nc.sync.dma_start(reps[:], rep_d)

            if X8:
                # dequant: xs = bf16(xq8) * xsc (per [chan, 128-token block])
                for s in range(CS):
                    nc.vector.tensor_copy(xs[s][:], xq8[s][:])
                    nc.vector.tensor_tensor(
                        xs[s][:].rearrange("w (b t) -> w b t", t=W),
                        xs[s][:].rearrange("w (b t) -> w b t", t=W),
                        xscs[s][:].unsqueeze(2).to_broadcast((W, NW, W)),
                        ALU.mult,
                    )

            # ones column of vt (col D of each [W, NW, H, D+1] slot)
            nc.vector.memset(vt[:, :, :, D], 1.0)

            # ---- B + C: projections ----
            with tc.tile_pool(name="projps", bufs=1, space="PSUM") as pps, \
                 tc.tile_pool(name="vps", bufs=2, space="PSUM") as vps, \
                 tc.tile_pool(name="ssqps", bufs=1, space="PSUM") as sps, \
                 tc.tile_pool(name="bcps", bufs=1, space="PSUM") as bps, \
                 tc.tile_pool(name="cscr", bufs=2) as cscr, \
                 tc.tile_pool(name="rnscr", bufs=4) as rnscr:

                # B: v projection, token-major
                for tt in range(NW):
                    pv = vps.tile([W, C], F32, name="vpsum")
                    for ks in range(CS):
                        nc.tensor.matmul(
                            pv[:],
                            xs[ks][:, tt * W:(tt + 1) * W],
                            wvs[ks][:],
                            start=(ks == 0), stop=(ks == CS - 1),
                        )
                    # copy [W, 512] -> vt[:, tt, :, 0:64] (stride D+1 per head)
                    nc.scalar.copy(vt[:, tt, :, 0:D], pv[:].rearrange("w (h d) -> w h d", d=D))

                # C: q, k channel-major + rmsnorm
                for t_idx, (off, dst) in enumerate([(0, qh), (C, kh)]):
                    for nt in range(NT):
                        nsl = slice(nt * C, (nt + 1) * C)
                        pq = pps.tile([W, CS, C], F32, name="projpsum")
                        for os in range(CS):
                            for ks in range(CS):
                                nc.tensor.matmul(
                                    pq[:, os, :],
                                    wqks[ks][:, off + os * W: off + (os + 1) * W],
                                    xs[ks][:, nsl],
                                    start=(ks == 0), stop=(ks == CS - 1),
                                )
                        # squares (bf16) for ssq matmul
                        q2 = cscr.tile([W, CS, C], BF16, name="q2")
                        for ks in range(CS):
                            nc.scalar.activation(q2[:, ks, :], pq[:, ks, :], AF.Square)
                        # ssq[h, tok] = blockdiag-ones^T @ q2
                        pssq = sps.tile([H, C], F32, name="ssqpsum")
                        for ks in range(CS):
                            nc.tensor.matmul(
                                pssq[:], bds[ks][:], q2[:, ks, :],
                                start=(ks == 0), stop=(ks == CS - 1),
                            )
                        # s = sqrt(ssq + eps); rn = 1/s (bf16)
                        s_sb = rnscr.tile([H, C], F32, name="s_sb")
                        nc.scalar.activation(s_sb[:], pssq[:], AF.Sqrt)
                        rn16 = rnscr.tile([H, C], BF16, name="rn16")
                        with nc.allow_low_precision(reason="rn broadcast in bf16"):
                            nc.vector.reciprocal(rn16[:], s_sb[:])
                        # broadcast rn to channels via PE repeat-matrix matmul
                        for s in range(CS):
                            rnbp = bps.tile([W, C], F32, name="rnbp")
                            nc.tensor.matmul(
                                rnbp[:], reps[:, s * W:(s + 1) * W], rn16[:],
                                start=True, stop=True,
                            )
                            rnb = rnscr.tile([W, C], BF16, name="rnb")
                            nc.vector.tensor_copy(rnb[:], rnbp[:])
                            if t_idx == 1:  # fold cs (=8*qs*ks per channel) into k's rn
                                nc.vector.tensor_scalar_mul(rnb[:], rnb[:], css[s][:])
                            nc.vector.tensor_tensor(
                                dst[s][:, nsl], pq[:, s, :], rnb[:], ALU.mult,
                            )

            # ---- D: attention ----
            with tc.tile_pool(name="sps2", bufs=2, space="PSUM") as scps, \
                 tc.tile_pool(name="pvps", bufs=4, space="PSUM") as pvps, \
                 tc.tile_pool(name="pscr", bufs=3) as pscr, \
                 tc.tile_pool(name="rcscr", bufs=4) as rcscr:
                for h in range(H):
                    s = h // 2
                    doff = D * (h % 2)
                    ksl = kh[s][doff:doff + D, :]
                    qsl = qh[s][doff:doff + D, :]
                    p_groups = []
                    for bg in range(4):  # block groups of 4
                        psc = scps.tile([W, 4, 2 * W], F32, name="scpsum")
                        for j in range(4):
                            b = 4 * bg + j
                            nq = min(2 * W, N - b * W)
                            nc.tensor.matmul(
                                psc[:, j, 0:nq],
                                ksl[:, b * W:(b + 1) * W],
                                qsl[:, b * W: b * W + nq],
                                start=True, stop=True,
                            )
                        p16 = pscr.tile([W, 4, 2 * W], BF16, name="p16")
                        nc.scalar.activation(p16[:, 0:2, :], psc[:, 0:2, :], AF.Exp)
                        nc.scalar.activation(p16[:, 2:4, :], psc[:, 2:4, :], AF.Exp)
                        nc.vector.tensor_tensor(
                            p16[:], p16[:],
                            mks[:].unsqueeze(1).to_broadcast((W, 4, 2 * W)),
                            ALU.mult,
                        )
                        p_groups.append(p16)

                    for wg in range(4):  # window groups of 4
                        ppv = pvps.tile([W, 4, D + 1], F32, name="pvpsum")
                        for wi in range(4):
                            w = 4 * wg + wi
                            mm_args = []
                            if w > 0:
                                bp, jp = (w - 1) // 4, (w - 1) % 4
                                mm_args.append(
                                    p_groups[bp][:, jp, W:2 * W])  # prev block right half
                            mm_args.append(
                                p_groups[w // 4][:, w % 4, 0:W])  # this block left half
                            for mi, lhsT in enumerate(mm_args):
                                nc.tensor.matmul(
                                    ppv[:, wi, :],
                                    lhsT,
                                    vt[:, w if mi == len(mm_args) - 1 else w - 1, h, :],
                                    start=(mi == 0), stop=(mi == len(mm_args) - 1),
                                )
                        rc = rcscr.tile([W, 4], F32, name="rc")
                        nc.vector.reciprocal(rc[:], ppv[:, :, D])
                        nc.vector.tensor_tensor(
                            att[:, 4 * wg:4 * wg + 4, h * D:(h + 1) * D],
                            ppv[:, :, 0:D],
                            rc[:].unsqueeze(2).to_broadcast((W, 4, D)),
                            ALU.mult,
                        )

            # ---- E: transpose att (token-major) -> attc (channel-major) ----
            for s in range(CS):
                for tt in range(NW):
                    nc.sync.dma_start(
                        attc[s][:, tt * W:(tt + 1) * W],
                        att[:, tt, s * W:(s + 1) * W],
                        transpose=True,
                    )

            # ---- F: output projection + int8 block quantization ----
            with tc.tile_pool(name="ops", bufs=1, space="PSUM") as ops, \
                 tc.tile_pool(name="oscr", bufs=2) as oscr, \
                 tc.tile_pool(name="qscr", bufs=4) as qscr:
                for nt in range(NT):
                    nsl = slice(nt * C, (nt + 1) * C)
                    po = ops.tile([W, CS, C], F32, name="outpsum")
                    for os in range(CS):
                        for ks in range(CS):
                            nc.tensor.matmul(
                                po[:, os, :],
                                wos[ks][:, os * W:(os + 1) * W],
                                attc[ks][:, nsl],
                                start=(ks == 0), stop=(ks == CS - 1),
                            )
                    if O8:
                        for os in range(CS):
                            pov = po[:, os, :].rearrange("w (b t) -> w b t", t=W)
                            amax = qscr.tile([W, 4], F32, name="amax")
                            nc.vector.tensor_reduce(
                                amax[:], pov, axis=AX.X, op=ALU.max,
                                apply_absolute_value=True)
                            # scale = amax/127 (guarded); rcp = 127/amax
                            nc.vector.tensor_scalar_max(amax[:], amax[:], 1e-30)
                            # scale stored in bf16; its rounding cancels in the
                            # roundtrip because rcp is derived from the stored
                            # bf16 value (q*osc == po up to the int8 grid)
                            osc_sb = qscr.tile([W, 4], BF16, name="osc_sb")
                            with nc.allow_low_precision(reason="osc roundtrip cancels"):
                                nc.vector.tensor_scalar_mul(
                                    osc_sb[:], amax[:], 1.0 / 127.0)
                            nc.sync.dma_start(
                                oq_d[os * W:(os + 1) * W,
                                     N + nt * 8:N + nt * 8 + 8].bitcast(BF16),
                                osc_sb[:])
                            rcp = qscr.tile([W, 4], F32, name="rcp")
                            nc.vector.reciprocal(rcp[:], osc_sb[:])
                            q8 = qscr.tile([W, 4, W], I8, name="q8")
                            nc.vector.tensor_tensor(
                                q8[:], pov,
                                rcp[:].unsqueeze(2).to_broadcast((W, 4, W)),
                                ALU.mult)
                            nc.sync.dma_start(oq_d[os * W:(os + 1) * W, nsl], q8[:])
                    else:
                        osb = oscr.tile([W, CS, C], BF16, name="osb")
                        for os in range(CS):
                            nc.scalar.copy(osb[:, os, :], po[:, os, :])
                        for os in range(CS):
                            nc.sync.dma_start(out_d[os * W:(os + 1) * W, nsl],
                                              osb[:, os, :])

    nc.compile()
    _CACHE["nc"] = nc
    return nc


def _host_prep(x, w_qkv, w_out, q_scale, k_scale):
    """FULL inputs -> host arrays (weights single-copy, x per-core)."""
    bf = ml_dtypes.bfloat16
    wqk = np.ascontiguousarray(w_qkv[: 2 * C].T).astype(bf)       # [C, 2C]
    wv = np.ascontiguousarray(w_qkv[2 * C:].T).astype(bf)         # [C, C]
    wo = np.ascontiguousarray(np.asarray(w_out).T).astype(bf)     # [C, C]
    cs = (8.0 * np.asarray(q_scale) * np.asarray(k_scale)).astype(np.float32)
    cs = np.tile(cs, H).reshape(C, 1)                             # [C, 1]
    bd = np.zeros((C, H), dtype=bf)
    for h in range(H):
        bd[h * D:(h + 1) * D, h] = 1.0
    i_idx = np.arange(2 * W)[None, :]
    j_idx = np.arange(W)[:, None]
    mk = np.where(
        i_idx < W, (j_idx <= i_idx), ((i_idx - W) <= j_idx)
    ).astype(bf)                                                   # [W, 2W]
    rep = np.ascontiguousarray(bd.T)                               # [H, C]
    weights = {"wqk": wqk, "wv": wv, "wo": wo, "cs": cs, "bd": bd,
               "mk": mk, "rep": rep}

    x = np.asarray(x, np.float32)
    if X8:
        v = x.reshape(B, C, NW, W)
        am = np.abs(v).max(axis=3, keepdims=True)
        # store scales in bf16 and quantize against the ROUNDED value so the
        # scale roundoff cancels in the device-side dequant
        scb = (np.maximum(am, 1e-30) / 127.0).astype(bf)
        xq = np.clip(np.round(v / scb.astype(np.float32)), -127, 127).astype(np.int8)
        pack = np.empty((B * C, N + 2 * NW), np.int8)
        pack[:, :N] = xq.reshape(B * C, N)
        pack[:, N:] = np.ascontiguousarray(
            scb.reshape(B * C, NW)).view(np.int8)
        xin = {"xq": pack}
    else:
        xin = {"x": x.astype(bf).reshape(B * C, N)}
    return xin, weights


def _get_runner():
    """Build (once) the cached jitted shard_map executable + device state."""
    if "runner" in _CACHE:
        return _CACHE["runner"]
    nc = build_nc()
    install_neuronx_cc_hook()

    partition_name = (nc.partition_id_tensor.name
                      if nc.partition_id_tensor is not None else None)
    in_names, out_names, out_avals = [], [], []
    for alloc in nc.m.functions[0].allocations:
        if not isinstance(alloc, mybir.MemoryLocationSet):
            continue
        name = alloc.memorylocations[0].name
        if alloc.kind == "ExternalInput":
            if name != partition_name:
                in_names.append(name)
        elif alloc.kind == "ExternalOutput":
            out_names.append(name)
            out_avals.append(jax.core.ShapedArray(
                tuple(alloc.tensor_shape), mybir.dt.np(alloc.dtype)))
    n_params = len(in_names)
    n_outs = len(out_names)
    all_names = in_names + out_names + ([partition_name] if partition_name else [])

    def _body(*args):
        operands = list(args)
        if partition_name is not None:
            operands.append(partition_id_tensor())
        return tuple(_bass_exec_p.bind(
            *operands, out_avals=tuple(out_avals), in_names=tuple(all_names),
            out_names=tuple(out_names), lowering_input_output_aliases=(),
            sim_require_finite=True, sim_require_nnan=True, nc=nc))

    devices = jax.devices()[:B]
    mesh = Mesh(np.asarray(devices), ("core",))
    sharding = NamedSharding(mesh, PartitionSpec("core"))
    donate = tuple(range(n_params, n_params + n_outs))
    sharded = jax.jit(
        shard_map(_body, mesh=mesh,
                  in_specs=(PartitionSpec("core"),) * (n_params + n_outs),
                  out_specs=(PartitionSpec("core"),) * n_outs,
                  check_rep=False),
        donate_argnums=donate, keep_unused=True)

    out_shapes = [(B * av.shape[0], *av.shape[1:]) for av in out_avals]
    out_dtypes = [av.dtype for av in out_avals]
    zeros_fn = jax.jit(
        lambda: tuple(jnp.zeros(s, d) for s, d in zip(out_shapes, out_dtypes)),
        out_shardings=(sharding,) * n_outs)

    runner = {
        "nc": nc, "sharded": sharded, "sharding": sharding,
        "in_names": in_names, "out_names": out_names,
        "zeros_fn": zeros_fn,
        "wkey": None, "wdev": None,   # memoized device-resident weights
        "prev_out": None,             # previous output buffers (donated next call)
    }
    _CACHE["runner"] = runner
    return runner


def _run_prepped(xin, weights):
    """Steady-state path: upload x, execute, fetch + dequantize -> f32 [B,C,N]."""
    r = _get_runner()

    if r.get("wobj") is weights:   # same dict object => same content, skip hash
        wkey = r["wkey"]
    else:
        wkey = hashlib.blake2b(
            b"".join(np.ascontiguousarray(weights[n]).tobytes()
                     for n in sorted(weights)),
            digest_size=16).digest()
        r["wobj"] = weights
    if r["wkey"] != wkey:
        wdev = {}
        for name, arr in weights.items():
            rep = np.ascontiguousarray(np.broadcast_to(
                arr, (B, *arr.shape)).reshape(B * arr.shape[0], *arr.shape[1:]))
            wdev[name] = jax.device_put(rep, r["sharding"])
        r["wdev"] = wdev
        r["wkey"] = wkey
        r["prev_out"] = None

    x_dev = jax.device_put(list(xin.values()), [r["sharding"]] * len(xin))
    x_dev = dict(zip(xin.keys(), x_dev))

    outs = r["prev_out"]
    if outs is None:
        outs = r["zeros_fn"]()
    args = [x_dev[n] if n in x_dev else r["wdev"][n] for n in r["in_names"]]
    res = r["sharded"](*args, *outs)
    r["prev_out"] = res

    if O8:
        packed_dev = res[r["out_names"].index("oq")]
        shards = list(packed_dev.addressable_shards)
        for sh in shards:
            sh.data.copy_to_host_async()
        # dequantize each core's shard while later shards are still in flight.
        # Ping-pong between two preallocated host buffers (avoids ~15 ms of
        # fresh-page faults per call); the result a caller holds stays valid
        # until two further kernel() calls have been made.
        bufs = r.setdefault("obufs", [np.zeros((B, C, NW, W), np.float32),
                                      np.zeros((B, C, NW, W), np.float32)])
        r["obuf_i"] = 1 - r.get("obuf_i", 1)
        out = bufs[r["obuf_i"]]
        for sh in shards:
            i = sh.index[0].start // C
            p = np.asarray(sh.data)                               # [C, N+32] i8
            osc = np.ascontiguousarray(p[:, N:]).view(
                ml_dtypes.bfloat16).astype(np.float32)             # [C, NW]
            np.multiply(p[:, :N].reshape(C, NW, W),
                        osc.reshape(C, NW, 1), out=out[i])
        return out.reshape(B, C, N)
    out = np.asarray(res[r["out_names"].index("out")])
    return out.astype(np.float32).reshape(B, C, N)


def kernel(x, w_qkv, w_out, q_scale, k_scale):
    x = np.asarray(x)
    assert x.shape == (B, C, N)
    xin, weights = _host_prep(x, w_qkv, w_out, q_scale, k_scale)
    return _run_prepped(xin, weights)
